# revision 1
# baseline (speedup 1.0000x reference)
"""Fused multi-head attention block (QKV proj + softmax attention + out-proj
+ LayerNorm) for Trainium2, sharded over 8 NeuronCores.

Sharding: tensor-parallel over heads. Core c owns heads [4c, 4c+4).
  - Each core computes q/k/v for its heads over the whole sequence
    (both batches), attention in S^T (keys-on-partitions) layout, and the
    unnormalized per-head attention output O^T [128ch, 4096 rows].
  - An AllToAll re-shards from head-parallel to row-parallel (512 rows per
    core); softmax denominators travel through a second small AllToAll and
    normalization happens after the exchange.
  - Each core then applies the full 1024x1024 output projection + bias +
    LayerNorm for its 512 rows.

dtypes: fp32 storage; QKV and S^T matmuls run as float32r (TF32-like, full
PE rate, rel err ~1.6e-4); exp(S^T), PV and out-proj use bf16.
"""
import sys

for _p in ("/opt/trn_rl_repo", "/root/.axon_site/_ro/trn_rl_repo"):
    if _p not in sys.path:
        sys.path.insert(0, _p)

import numpy as np

import concourse.bass as bass
import concourse.tile as tile
from concourse import bacc, mybir
from concourse.masks import make_identity

F32 = mybir.dt.float32
F32R = mybir.dt.float32r
BF16 = mybir.dt.bfloat16
AF = mybir.ActivationFunctionType
ALU = mybir.AluOpType

N_CORES = 8
B, N, DIM = 2, 2048, 1024
HEADS, DH = 32, 32           # 32 heads x 32 dim/head
HPC = HEADS // N_CORES       # 4 heads per core
ROWS = B * N                 # 4096 global rows
RPC = ROWS // N_CORES        # 512 rows per core
SCALE = DH ** -0.5
EPS = 1e-6
KT = N // 128                # 16 key tiles per batch
QB = 512                     # q-block width
NQB = N // QB                # 4 q-blocks per batch
RC = 256                     # projection row-chunk
NRC = ROWS // RC             # 16 row chunks


def _build(debug=False):
    nc = bacc.Bacc("TRN2", target_bir_lowering=False, debug=False,
                   num_devices=N_CORES)

    xT_d = nc.dram_tensor("xT", [DIM, ROWS], F32R, kind="ExternalInput").ap()
    wqkv_d = nc.dram_tensor("wqkv", [DIM, 3 * HPC * DH], F32R,
                            kind="ExternalInput").ap()
    wout_d = nc.dram_tensor("wout", [DIM, DIM], F32, kind="ExternalInput").ap()
    bout_d = nc.dram_tensor("bout", [DIM], F32, kind="ExternalInput").ap()
    gamma_d = nc.dram_tensor("gamma", [DIM], F32, kind="ExternalInput").ap()
    beta_d = nc.dram_tensor("beta", [DIM], F32, kind="ExternalInput").ap()
    out_d = nc.dram_tensor("out", [RPC, DIM], F32, kind="ExternalOutput").ap()
    if debug:
        dbg_qT = nc.dram_tensor("dbg_qT", [128, ROWS], F32,
                                kind="ExternalOutput").ap()
        dbg_kT = nc.dram_tensor("dbg_kT", [128, ROWS], F32,
                                kind="ExternalOutput").ap()
        dbg_V = nc.dram_tensor("dbg_V", [128, 2 * KT, 128], BF16,
                               kind="ExternalOutput").ap()
        dbg_exp = nc.dram_tensor("dbg_exp", [128, KT, 2, QB], BF16,
                                 kind="ExternalOutput").ap()
        dbg_att = nc.dram_tensor("dbg_att", [N_CORES, 128, RPC], BF16,
                                 kind="ExternalOutput").ap()
        dbg_den = nc.dram_tensor("dbg_den", [HEADS, RPC], F32,
                                 kind="ExternalOutput").ap()
        dbg_a2a = nc.dram_tensor("dbg_a2a", [128, N_CORES, RPC], BF16,
                                 kind="ExternalOutput").ap()

    with tile.TileContext(nc) as tc:
        with (
            tc.tile_pool(name="const", bufs=1) as const,
            tc.tile_pool(name="work", bufs=1) as work,
            tc.tile_pool(name="ps", bufs=1, space="PSUM") as ps,
            tc.tile_pool(name="dram", bufs=1, space="DRAM") as dram,
        ):
            # ---------------- constants / weights ----------------
            wqkv_sb = const.tile([128, 8, 3 * HPC * DH], F32R)
            nc.sync.dma_start(
                wqkv_sb[:], wqkv_d.rearrange("(kc p) m -> p kc m", p=128))
            ones_bf = const.tile([128, 1], BF16)
            nc.vector.memset(ones_bf[:], 1.0)
            ident = const.tile([128, 128], F32)
            make_identity(nc, ident[:])
            eps_sb = const.tile([128, 1], F32)
            nc.vector.memset(eps_sb[:], EPS)
            # row-broadcast vectors [128, 1024]
            bout_bc = const.tile([128, DIM], F32)
            nc.gpsimd.dma_start(out=bout_bc[:], in_=bass.AP(
                tensor=bout_d.tensor, offset=bout_d.offset,
                ap=[[0, 128], [1, DIM]]))
            gamma_bc = const.tile([128, DIM], F32)
            nc.gpsimd.dma_start(out=gamma_bc[:], in_=bass.AP(
                tensor=gamma_d.tensor, offset=gamma_d.offset,
                ap=[[0, 128], [1, DIM]]))
            beta_bc = const.tile([128, DIM], F32)
            nc.gpsimd.dma_start(out=beta_bc[:], in_=bass.AP(
                tensor=beta_d.tensor, offset=beta_d.offset,
                ap=[[0, 128], [1, DIM]]))
            # w_out -> bf16 [128, 8, 1024]
            wout_bf = const.tile([128, 8, DIM], BF16)

            # ---------------- persistent activations ----------------
            qT_sb = const.tile([128, ROWS], F32R)   # 4h x 32d on partitions
            kT_sb = const.tile([128, ROWS], F32R)
            V_sb = const.tile([128, 2 * KT, 128], BF16)  # [key%128, ktile, ch]
            expA = const.tile([128, KT, 2, QB], BF16)    # heads 0,1
            expB = const.tile([128, KT, 2, QB], BF16)    # heads 2,3

            # ---------------- dram bounce buffers ----------------
            a2a_in = dram.tile([N_CORES, 128, RPC], BF16)
            a2a_out = dram.tile([N_CORES, 128, RPC], BF16)
            d2a_in = dram.tile([N_CORES, HPC, RPC], F32)
            d2a_out = dram.tile([N_CORES, HPC, RPC], F32)
            recip_dram = dram.tile([HEADS, RPC], BF16)

            # ---------------- phase A: projections ----------------
            def proj_rowchunk(rc):
                xt = work.tile([128, 8, RC], F32R, tag="xt", bufs=2,
                               name=f"xt_{rc}")
                nc.sync.dma_start(
                    xt[:],
                    xT_d[:, rc * RC:(rc + 1) * RC]
                    .rearrange("(kc p) n -> p kc n", p=128))
                for name, mofs, dst in (("q", 0, qT_sb), ("k", 128, kT_sb)):
                    pp = ps.tile([128, RC], F32, tag="aux",
                                 name=f"pp_{name}_{rc}")
                    for kc in range(8):
                        nc.tensor.matmul(
                            pp[:], wqkv_sb[:, kc, mofs:mofs + 128],
                            xt[:, kc, :], start=(kc == 0), stop=(kc == 7))
                    nc.vector.tensor_copy(dst[:, rc * RC:(rc + 1) * RC], pp[:])
                # v: project (vT layout), cast bf16, DMA-transpose into V_sb
                pv_ = ps.tile([128, RC], F32, tag="aux", name=f"pp_v_{rc}")
                for kc in range(8):
                    nc.tensor.matmul(
                        pv_[:], wqkv_sb[:, kc, 256:384], xt[:, kc, :],
                        start=(kc == 0), stop=(kc == 7))
                vt = work.tile([128, RC], F32, tag="vt", bufs=2,
                               name=f"vt_{rc}")
                nc.vector.tensor_copy(vt[:], pv_[:])
                for i in range(RC // 128):
                    tp = ps.tile([128, 128], F32, tag="aux",
                                 name=f"tp_{rc}_{i}")
                    nc.tensor.transpose(
                        tp[:], vt[:, i * 128:(i + 1) * 128], ident[:])
                    nc.vector.tensor_copy(
                        V_sb[:, rc * (RC // 128) + i, :], tp[:])

            def load_wout(j):
                st = work.tile([128, DIM], F32, tag="wstage", bufs=2,
                               name=f"wst_{j}")
                nc.sync.dma_start(st[:], wout_d[j * 128:(j + 1) * 128, :])
                nc.vector.tensor_copy(wout_bf[:, j, :], st[:])

            for rc in range(NRC // 2):      # batch-0 rows
                proj_rowchunk(rc)

            # ---------------- phase B: attention ----------------
            def attention_qblock(b, qb):
                q0 = b * N + qb * QB
                qsl = qT_sb[:, q0:q0 + QB]
                for kt in range(KT):
                    ksl = kT_sb[:, b * N + kt * 128: b * N + kt * 128 + 128]
                    pA = ps.tile([128, 2, QB], F32, tag="spA",
                                 name=f"pA_{b}_{qb}_{kt}")
                    pB = ps.tile([128, 2, QB], F32, tag="spB",
                                 name=f"pB_{b}_{qb}_{kt}")
                    for h in range(4):
                        dst = pA if h < 2 else pB
                        nc.tensor.matmul(
                            dst[:, h % 2, :],
                            ksl[32 * h:32 * h + 32, :],
                            qsl[32 * h:32 * h + 32, :],
                            start=True, stop=True, tile_position=(32 * h, 0))
                    nc.scalar.activation(expA[:, kt, :, :], pA[:], AF.Exp,
                                         scale=SCALE)
                    nc.scalar.activation(expB[:, kt, :, :], pB[:], AF.Exp,
                                         scale=SCALE)
                pvp = ps.tile([128, QB], F32, tag="pvt", bufs=2,
                              name=f"pv_{b}_{qb}")
                dnp = ps.tile([128, QB], F32, tag="dn", name=f"dn_{b}_{qb}")
                for kt in range(KT):
                    for h in range(4):
                        e = expA if h < 2 else expB
                        rhs = e[:, kt, h % 2, :]
                        nc.tensor.matmul(
                            pvp[32 * h:32 * h + 32, :],
                            V_sb[:, b * KT + kt, 32 * h:32 * h + 32],
                            rhs, start=(kt == 0), stop=(kt == KT - 1),
                            tile_position=(0, 32 * h))
                        nc.tensor.matmul(
                            dnp[32 * h:32 * h + 1, :],
                            ones_bf[:], rhs,
                            start=(kt == 0), stop=(kt == KT - 1),
                            tile_position=(0, 32 * h))
                att = work.tile([128, QB], BF16, tag="att", bufs=2,
                                name=f"att_{b}_{qb}")
                nc.vector.tensor_copy(att[:], pvp[:])
                nc.sync.dma_start(a2a_in[b * NQB + qb], att[:])
                dnsb = work.tile([128, QB], F32, tag="dnsb", bufs=2,
                                 name=f"dnsb_{b}_{qb}")
                for h in range(4):
                    nc.vector.tensor_copy(dnsb[32 * h:32 * h + 1, :],
                                          dnp[32 * h:32 * h + 1, :])
                nc.sync.dma_start(
                    d2a_in[b * NQB + qb],
                    dnsb[:].rearrange("(a c) q -> a c q", c=32)[:, 0, :])

            for qb in range(NQB):           # batch 0 + leftover projections
                attention_qblock(0, qb)
                proj_rowchunk(NRC // 2 + 2 * qb)
                proj_rowchunk(NRC // 2 + 2 * qb + 1)
                load_wout(2 * qb)
                load_wout(2 * qb + 1)
            for qb in range(NQB):
                attention_qblock(1, qb)

            if debug:
                nc.sync.dma_start(dbg_qT, qT_sb[:].bitcast(F32))
                nc.sync.dma_start(dbg_kT, kT_sb[:].bitcast(F32))
                nc.sync.dma_start(dbg_V, V_sb[:])
                nc.sync.dma_start(dbg_exp, expB[:])

            # ---------------- phase C: exchange + out-proj + LN ----------
            nc.gpsimd.collective_compute(
                "AllToAll", ALU.bypass, replica_groups=[list(range(N_CORES))],
                ins=[d2a_in.opt()], outs=[d2a_out.opt()])
            nc.gpsimd.collective_compute(
                "AllToAll", ALU.bypass, replica_groups=[list(range(N_CORES))],
                ins=[a2a_in.opt()], outs=[a2a_out.opt()])

            a2a_sb = const.tile([128, N_CORES, RPC], BF16)
            for j in range(N_CORES):
                nc.sync.dma_start(a2a_sb[:, j, :], a2a_out[j])
            denT = work.tile([HEADS, RPC], F32, tag="denT")
            for j in range(N_CORES):
                nc.sync.dma_start(denT[HPC * j:HPC * j + HPC, :], d2a_out[j])
            recip = work.tile([HEADS, RPC], F32, tag="recip")
            nc.vector.reciprocal_approx_fast(out=recip[:], in_=denT[:])
            recip_bf = work.tile([HEADS, RPC], BF16, tag="recip_bf")
            nc.vector.tensor_copy(recip_bf[:], recip[:])
            nc.sync.dma_start(recip_dram[:], recip_bf[:])
            recip_bc = const.tile([128, N_CORES, RPC], BF16)
            for s in range(4):
                src2 = bass.AP(
                    tensor=recip_dram.tensor,
                    offset=recip_dram.offset + s * RPC,
                    ap=[[0, 32], [HPC * RPC, N_CORES], [1, RPC]])
                nc.gpsimd.dma_start(
                    out=recip_bc[32 * s:32 * s + 32, :, :], in_=src2)
            nc.vector.tensor_tensor(a2a_sb[:], a2a_sb[:], recip_bc[:],
                                    ALU.mult)
            if debug:
                nc.sync.dma_start(dbg_a2a, a2a_sb[:])

            op_tags = ("pvt", "dn", "aux", "pvt")
            for mt in range(RPC // 128):
                osb = work.tile([128, DIM], F32, tag="osb", bufs=2,
                                name=f"osb_{mt}")
                for nb in range(2):
                    _tag = op_tags[(2 * mt + nb) % 4]
                    op = ps.tile([128, 512], F32, tag=_tag,
                                 bufs=(2 if _tag == "pvt" else 1),
                                 name=f"op_{mt}_{nb}")
                    for j in range(N_CORES):
                        nc.tensor.matmul(
                            op[:], a2a_sb[:, j, mt * 128:(mt + 1) * 128],
                            wout_bf[:, j, nb * 512:(nb + 1) * 512],
                            start=(j == 0), stop=(j == N_CORES - 1))
                    nc.vector.tensor_tensor(
                        osb[:, nb * 512:(nb + 1) * 512], op[:],
                        bout_bc[:, nb * 512:(nb + 1) * 512], ALU.add)
                # LayerNorm over the 1024 free dim
                stats = work.tile([128, 2, 6], F32, tag="stats", bufs=2,
                                  name=f"stats_{mt}")
                for sg in range(2):
                    nc.vector.bn_stats(out=stats[:, sg, :],
                                       in_=osb[:, sg * 512:(sg + 1) * 512])
                mv = work.tile([128, 2], F32, tag="mv", bufs=2,
                               name=f"mv_{mt}")
                nc.vector.bn_aggr(out=mv[:], in_=stats[:])
                rstd = work.tile([128, 1], F32, tag="rstd", bufs=2,
                                 name=f"rstd_{mt}")
                nc.scalar.activation(out=rstd[:], in_=mv[:, 1:2], func=AF.Sqrt,
                                     bias=eps_sb[:], scale=1.0)
                nc.vector.reciprocal(out=rstd[:], in_=rstd[:])
                nc.vector.tensor_scalar(
                    out=osb[:], in0=osb[:], scalar1=mv[:, 0:1],
                    scalar2=rstd[:], op0=ALU.subtract, op1=ALU.mult)
                nc.vector.tensor_tensor(osb[:], osb[:], gamma_bc[:], ALU.mult)
                nc.vector.tensor_tensor(osb[:], osb[:], beta_bc[:], ALU.add)
                nc.sync.dma_start(out_d[mt * 128:(mt + 1) * 128, :], osb[:])

    nc.compile()
    return nc


class _Runner:
    """Compile once; run the SPMD kernel on 8 cores via PJRT repeatedly."""

    def __init__(self):
        self.nc = _build()
        import jax
        from jax.sharding import Mesh, PartitionSpec
        from jax.experimental.shard_map import shard_map
        from concourse import bass2jax
        bass2jax.install_neuronx_cc_hook()

        nc = self.nc
        part_name = (nc.partition_id_tensor.name
                     if nc.partition_id_tensor else None)
        in_names, out_names, out_avals = [], [], []
        for alloc in nc.m.functions[0].allocations:
            if not isinstance(alloc, mybir.MemoryLocationSet):
                continue
            name = alloc.memorylocations[0].name
            if alloc.kind == "ExternalInput":
                if name != part_name:
                    in_names.append(name)
            elif alloc.kind == "ExternalOutput":
                out_names.append(name)
                out_avals.append(jax.core.ShapedArray(
                    tuple(alloc.tensor_shape), mybir.dt.np(alloc.dtype)))
        self.in_names = list(in_names)
        self.out_names = out_names
        self.out_avals = out_avals
        all_in_names = in_names + out_names
        if part_name is not None:
            all_in_names = all_in_names + [part_name]

        def _body(*args):
            operands = list(args)
            if part_name is not None:
                operands.append(bass2jax.partition_id_tensor())
            outs = bass2jax._bass_exec_p.bind(
                *operands, out_avals=tuple(out_avals),
                in_names=tuple(all_in_names), out_names=tuple(out_names),
                lowering_input_output_aliases=(),
                sim_require_finite=True, sim_require_nnan=True, nc=nc)
            return tuple(outs)

        devices = jax.devices()[:N_CORES]
        mesh = Mesh(np.asarray(devices), ("core",))
        nin = len(self.in_names) + len(out_names)
        self.fn = jax.jit(shard_map(
            _body, mesh=mesh, in_specs=(PartitionSpec("core"),) * nin,
            out_specs=(PartitionSpec("core"),) * len(out_names),
            check_rep=False))
        self.jax = jax

    def stage(self, in_maps):
        """Concatenate per-core inputs + zero outputs, device_put once."""
        concat = [np.concatenate([m[name] for m in in_maps], axis=0)
                  for name in self.in_names]
        zeros = [np.zeros((N_CORES * a.shape[0], *a.shape[1:]), a.dtype)
                 for a in self.out_avals]
        return [self.jax.device_put(x) for x in concat + zeros]

    def run_staged(self, staged):
        outs = self.fn(*staged)
        self.jax.block_until_ready(outs)
        return outs

    def run(self, in_maps):
        outs = self.run_staged(self.stage(in_maps))
        return [
            {name: np.asarray(outs[i]).reshape(
                N_CORES, *self.out_avals[i].shape)[c]
             for i, name in enumerate(self.out_names)}
            for c in range(N_CORES)
        ]


_RUNNER = None


def _get_runner():
    global _RUNNER
    if _RUNNER is None:
        _RUNNER = _Runner()
    return _RUNNER


def _make_in_maps(x, w_qkv, w_out, b_out, ln_gamma, ln_beta):
    x = np.asarray(x, dtype=np.float32)
    w_qkv = np.asarray(w_qkv, dtype=np.float32)
    w_out = np.asarray(w_out, dtype=np.float32)
    b_out = np.asarray(b_out, dtype=np.float32)
    ln_gamma = np.asarray(ln_gamma, dtype=np.float32)
    ln_beta = np.asarray(ln_beta, dtype=np.float32)

    xT = np.ascontiguousarray(x.reshape(ROWS, DIM).T)
    in_maps = []
    for c in range(N_CORES):
        h0 = HPC * c * DH
        cols = np.concatenate([
            w_qkv[:, h0:h0 + HPC * DH],
            w_qkv[:, DIM + h0:DIM + h0 + HPC * DH],
            w_qkv[:, 2 * DIM + h0:2 * DIM + h0 + HPC * DH],
        ], axis=1)
        in_maps.append({
            "xT": xT,
            "wqkv": np.ascontiguousarray(cols),
            "wout": w_out,
            "bout": b_out,
            "gamma": ln_gamma,
            "beta": ln_beta,
        })
    return in_maps


def kernel(x, w_qkv, w_out, b_out, ln_gamma, ln_beta):
    runner = _get_runner()
    in_maps = _make_in_maps(x, w_qkv, w_out, b_out, ln_gamma, ln_beta)
    results = runner.run(in_maps)
    out = np.concatenate([results[c]["out"] for c in range(N_CORES)], axis=0)
    return out.reshape(B, N, DIM).astype(np.float32)



# revision 2
# speedup vs baseline: 223.1534x; 223.1534x over previous
"""Fused multi-head attention block (QKV proj + softmax attention + out-proj
+ LayerNorm) for Trainium2, sharded over 8 NeuronCores.

Sharding: tensor-parallel over heads. Core c owns heads [4c, 4c+4).
  - Each core computes q/k/v for its heads over the whole sequence
    (both batches), attention in S^T (keys-on-partitions) layout, and the
    unnormalized per-head attention output O^T [128ch, 4096 rows].
  - An AllToAll re-shards from head-parallel to row-parallel (512 rows per
    core); softmax denominators travel through a second small AllToAll and
    normalization happens after the exchange.
  - Each core then applies the full 1024x1024 output projection + bias +
    LayerNorm for its 512 rows.

dtypes: fp32 storage; QKV and S^T matmuls run as float32r (TF32-like, full
PE rate, rel err ~1.6e-4); exp(S^T), PV and out-proj use bf16.
"""
import sys

for _p in ("/opt/trn_rl_repo", "/root/.axon_site/_ro/trn_rl_repo"):
    if _p not in sys.path:
        sys.path.insert(0, _p)

import numpy as np

import concourse.bass as bass
import concourse.tile as tile
from concourse import bacc, mybir
from concourse.masks import make_identity

F32 = mybir.dt.float32
F32R = mybir.dt.float32r
BF16 = mybir.dt.bfloat16
AF = mybir.ActivationFunctionType
ALU = mybir.AluOpType

N_CORES = 8
B, N, DIM = 2, 2048, 1024
HEADS, DH = 32, 32           # 32 heads x 32 dim/head
HPC = HEADS // N_CORES       # 4 heads per core
ROWS = B * N                 # 4096 global rows
RPC = ROWS // N_CORES        # 512 rows per core
SCALE = DH ** -0.5
EPS = 1e-6
KT = N // 128                # 16 key tiles per batch
QB = 512                     # q-block width
NQB = N // QB                # 4 q-blocks per batch
RC = 256                     # projection row-chunk
NRC = ROWS // RC             # 16 row chunks


def _build(debug=False):
    nc = bacc.Bacc("TRN2", target_bir_lowering=False, debug=False,
                   num_devices=N_CORES)

    xT_d = nc.dram_tensor("xT", [DIM, ROWS], F32R, kind="ExternalInput").ap()
    wqkv_d = nc.dram_tensor("wqkv", [DIM, 3 * HPC * DH], F32R,
                            kind="ExternalInput").ap()
    wout_d = nc.dram_tensor("wout", [DIM, DIM], F32, kind="ExternalInput").ap()
    bout_d = nc.dram_tensor("bout", [DIM], F32, kind="ExternalInput").ap()
    gamma_d = nc.dram_tensor("gamma", [DIM], F32, kind="ExternalInput").ap()
    beta_d = nc.dram_tensor("beta", [DIM], F32, kind="ExternalInput").ap()
    out_d = nc.dram_tensor("out", [RPC, DIM], F32, kind="ExternalOutput").ap()
    if debug:
        dbg_qT = nc.dram_tensor("dbg_qT", [128, ROWS], F32,
                                kind="ExternalOutput").ap()
        dbg_kT = nc.dram_tensor("dbg_kT", [128, ROWS], F32,
                                kind="ExternalOutput").ap()
        dbg_V = nc.dram_tensor("dbg_V", [128, 2 * KT, 128], BF16,
                               kind="ExternalOutput").ap()
        dbg_exp = nc.dram_tensor("dbg_exp", [128, KT, 2, QB], BF16,
                                 kind="ExternalOutput").ap()
        dbg_att = nc.dram_tensor("dbg_att", [N_CORES, 128, RPC], BF16,
                                 kind="ExternalOutput").ap()
        dbg_den = nc.dram_tensor("dbg_den", [HEADS, RPC], F32,
                                 kind="ExternalOutput").ap()
        dbg_a2a = nc.dram_tensor("dbg_a2a", [128, N_CORES, RPC], BF16,
                                 kind="ExternalOutput").ap()

    with tile.TileContext(nc) as tc:
        with (
            tc.tile_pool(name="const", bufs=1) as const,
            tc.tile_pool(name="work", bufs=1) as work,
            tc.tile_pool(name="ps", bufs=1, space="PSUM") as ps,
            tc.tile_pool(name="dram", bufs=1, space="DRAM") as dram,
        ):
            # ---------------- constants / weights ----------------
            wqkv_sb = const.tile([128, 8, 3 * HPC * DH], F32R)
            nc.sync.dma_start(
                wqkv_sb[:], wqkv_d.rearrange("(kc p) m -> p kc m", p=128))
            ones_bf = const.tile([128, 1], BF16)
            nc.vector.memset(ones_bf[:], 1.0)
            ident = const.tile([128, 128], F32)
            make_identity(nc, ident[:])
            eps_sb = const.tile([128, 1], F32)
            nc.vector.memset(eps_sb[:], EPS)
            # row-broadcast vectors [128, 1024]
            bout_bc = const.tile([128, DIM], F32)
            nc.gpsimd.dma_start(out=bout_bc[:], in_=bass.AP(
                tensor=bout_d.tensor, offset=bout_d.offset,
                ap=[[0, 128], [1, DIM]]))
            gamma_bc = const.tile([128, DIM], F32)
            nc.gpsimd.dma_start(out=gamma_bc[:], in_=bass.AP(
                tensor=gamma_d.tensor, offset=gamma_d.offset,
                ap=[[0, 128], [1, DIM]]))
            beta_bc = const.tile([128, DIM], F32)
            nc.gpsimd.dma_start(out=beta_bc[:], in_=bass.AP(
                tensor=beta_d.tensor, offset=beta_d.offset,
                ap=[[0, 128], [1, DIM]]))
            # w_out -> bf16 [128, 8, 1024]
            wout_bf = const.tile([128, 8, DIM], BF16)

            # ---------------- persistent activations ----------------
            qT_sb = const.tile([128, ROWS], F32R)   # 4h x 32d on partitions
            kT_sb = const.tile([128, ROWS], F32R)
            V_sb = const.tile([128, 2 * KT, 128], BF16)  # [key%128, ktile, ch]
            expA = const.tile([128, KT, 2, QB], BF16)    # heads 0,1
            expB = const.tile([128, KT, 2, QB], BF16)    # heads 2,3

            # ---------------- dram bounce buffers ----------------
            a2a_in = dram.tile([N_CORES, 128, RPC], BF16)
            a2a_out = dram.tile([N_CORES, 128, RPC], BF16)
            d2a_in = dram.tile([N_CORES, HPC, RPC], F32)
            d2a_out = dram.tile([N_CORES, HPC, RPC], F32)
            recip_dram = dram.tile([HEADS, RPC], BF16)

            # ---------------- phase A: projections ----------------
            def proj_rowchunk(rc):
                xt = work.tile([128, 8, RC], F32R, tag="xt", bufs=2,
                               name=f"xt_{rc}")
                nc.sync.dma_start(
                    xt[:],
                    xT_d[:, rc * RC:(rc + 1) * RC]
                    .rearrange("(kc p) n -> p kc n", p=128))
                for name, mofs, dst in (("q", 0, qT_sb), ("k", 128, kT_sb)):
                    pp = ps.tile([128, RC], F32, tag="aux",
                                 name=f"pp_{name}_{rc}")
                    for kc in range(8):
                        nc.tensor.matmul(
                            pp[:], wqkv_sb[:, kc, mofs:mofs + 128],
                            xt[:, kc, :], start=(kc == 0), stop=(kc == 7))
                    nc.vector.tensor_copy(dst[:, rc * RC:(rc + 1) * RC], pp[:])
                # v: project (vT layout), cast bf16, DMA-transpose into V_sb
                pv_ = ps.tile([128, RC], F32, tag="aux", name=f"pp_v_{rc}")
                for kc in range(8):
                    nc.tensor.matmul(
                        pv_[:], wqkv_sb[:, kc, 256:384], xt[:, kc, :],
                        start=(kc == 0), stop=(kc == 7))
                vt = work.tile([128, RC], F32, tag="vt", bufs=2,
                               name=f"vt_{rc}")
                nc.vector.tensor_copy(vt[:], pv_[:])
                for i in range(RC // 128):
                    tp = ps.tile([128, 128], F32, tag="aux",
                                 name=f"tp_{rc}_{i}")
                    nc.tensor.transpose(
                        tp[:], vt[:, i * 128:(i + 1) * 128], ident[:])
                    nc.vector.tensor_copy(
                        V_sb[:, rc * (RC // 128) + i, :], tp[:])

            def load_wout(j):
                st = work.tile([128, DIM], F32, tag="wstage", bufs=2,
                               name=f"wst_{j}")
                nc.sync.dma_start(st[:], wout_d[j * 128:(j + 1) * 128, :])
                nc.vector.tensor_copy(wout_bf[:, j, :], st[:])

            for rc in range(NRC // 2):      # batch-0 rows
                proj_rowchunk(rc)

            # ---------------- phase B: attention ----------------
            def attention_qblock(b, qb):
                q0 = b * N + qb * QB
                qsl = qT_sb[:, q0:q0 + QB]
                for kt in range(KT):
                    ksl = kT_sb[:, b * N + kt * 128: b * N + kt * 128 + 128]
                    pA = ps.tile([128, 2, QB], F32, tag="spA",
                                 name=f"pA_{b}_{qb}_{kt}")
                    pB = ps.tile([128, 2, QB], F32, tag="spB",
                                 name=f"pB_{b}_{qb}_{kt}")
                    for h in range(4):
                        dst = pA if h < 2 else pB
                        nc.tensor.matmul(
                            dst[:, h % 2, :],
                            ksl[32 * h:32 * h + 32, :],
                            qsl[32 * h:32 * h + 32, :],
                            start=True, stop=True, tile_position=(32 * h, 0))
                    nc.scalar.activation(expA[:, kt, :, :], pA[:], AF.Exp,
                                         scale=SCALE)
                    nc.scalar.activation(expB[:, kt, :, :], pB[:], AF.Exp,
                                         scale=SCALE)
                pvp = ps.tile([128, QB], F32, tag="pvt", bufs=2,
                              name=f"pv_{b}_{qb}")
                dnp = ps.tile([128, QB], F32, tag="dn", name=f"dn_{b}_{qb}")
                for kt in range(KT):
                    for h in range(4):
                        e = expA if h < 2 else expB
                        rhs = e[:, kt, h % 2, :]
                        nc.tensor.matmul(
                            pvp[32 * h:32 * h + 32, :],
                            V_sb[:, b * KT + kt, 32 * h:32 * h + 32],
                            rhs, start=(kt == 0), stop=(kt == KT - 1),
                            tile_position=(0, 32 * h))
                        nc.tensor.matmul(
                            dnp[32 * h:32 * h + 1, :],
                            ones_bf[:], rhs,
                            start=(kt == 0), stop=(kt == KT - 1),
                            tile_position=(0, 32 * h))
                att = work.tile([128, QB], BF16, tag="att", bufs=2,
                                name=f"att_{b}_{qb}")
                nc.vector.tensor_copy(att[:], pvp[:])
                nc.sync.dma_start(a2a_in[b * NQB + qb], att[:])
                dnsb = work.tile([128, QB], F32, tag="dnsb", bufs=2,
                                 name=f"dnsb_{b}_{qb}")
                for h in range(4):
                    nc.vector.tensor_copy(dnsb[32 * h:32 * h + 1, :],
                                          dnp[32 * h:32 * h + 1, :])
                nc.sync.dma_start(
                    d2a_in[b * NQB + qb],
                    dnsb[:].rearrange("(a c) q -> a c q", c=32)[:, 0, :])

            for qb in range(NQB):           # batch 0 + leftover projections
                attention_qblock(0, qb)
                proj_rowchunk(NRC // 2 + 2 * qb)
                proj_rowchunk(NRC // 2 + 2 * qb + 1)
                load_wout(2 * qb)
                load_wout(2 * qb + 1)
            for qb in range(NQB):
                attention_qblock(1, qb)

            if debug:
                nc.sync.dma_start(dbg_qT, qT_sb[:].bitcast(F32))
                nc.sync.dma_start(dbg_kT, kT_sb[:].bitcast(F32))
                nc.sync.dma_start(dbg_V, V_sb[:])
                nc.sync.dma_start(dbg_exp, expB[:])

            # ---------------- phase C: exchange + out-proj + LN ----------
            nc.gpsimd.collective_compute(
                "AllToAll", ALU.bypass, replica_groups=[list(range(N_CORES))],
                ins=[d2a_in.opt()], outs=[d2a_out.opt()])
            nc.gpsimd.collective_compute(
                "AllToAll", ALU.bypass, replica_groups=[list(range(N_CORES))],
                ins=[a2a_in.opt()], outs=[a2a_out.opt()])

            a2a_sb = const.tile([128, N_CORES, RPC], BF16)
            for j in range(N_CORES):
                nc.sync.dma_start(a2a_sb[:, j, :], a2a_out[j])
            denT = work.tile([HEADS, RPC], F32, tag="denT")
            for j in range(N_CORES):
                nc.sync.dma_start(denT[HPC * j:HPC * j + HPC, :], d2a_out[j])
            recip = work.tile([HEADS, RPC], F32, tag="recip")
            nc.vector.reciprocal_approx_fast(out=recip[:], in_=denT[:])
            recip_bf = work.tile([HEADS, RPC], BF16, tag="recip_bf")
            nc.vector.tensor_copy(recip_bf[:], recip[:])
            nc.sync.dma_start(recip_dram[:], recip_bf[:])
            recip_bc = const.tile([128, N_CORES, RPC], BF16)
            for s in range(4):
                src2 = bass.AP(
                    tensor=recip_dram.tensor,
                    offset=recip_dram.offset + s * RPC,
                    ap=[[0, 32], [HPC * RPC, N_CORES], [1, RPC]])
                nc.gpsimd.dma_start(
                    out=recip_bc[32 * s:32 * s + 32, :, :], in_=src2)
            nc.vector.tensor_tensor(a2a_sb[:], a2a_sb[:], recip_bc[:],
                                    ALU.mult)
            if debug:
                nc.sync.dma_start(dbg_a2a, a2a_sb[:])

            op_tags = ("pvt", "dn", "aux", "pvt")
            for mt in range(RPC // 128):
                osb = work.tile([128, DIM], F32, tag="osb", bufs=2,
                                name=f"osb_{mt}")
                for nb in range(2):
                    _tag = op_tags[(2 * mt + nb) % 4]
                    op = ps.tile([128, 512], F32, tag=_tag,
                                 bufs=(2 if _tag == "pvt" else 1),
                                 name=f"op_{mt}_{nb}")
                    for j in range(N_CORES):
                        nc.tensor.matmul(
                            op[:], a2a_sb[:, j, mt * 128:(mt + 1) * 128],
                            wout_bf[:, j, nb * 512:(nb + 1) * 512],
                            start=(j == 0), stop=(j == N_CORES - 1))
                    nc.vector.tensor_tensor(
                        osb[:, nb * 512:(nb + 1) * 512], op[:],
                        bout_bc[:, nb * 512:(nb + 1) * 512], ALU.add)
                # LayerNorm over the 1024 free dim
                stats = work.tile([128, 2, 6], F32, tag="stats", bufs=2,
                                  name=f"stats_{mt}")
                for sg in range(2):
                    nc.vector.bn_stats(out=stats[:, sg, :],
                                       in_=osb[:, sg * 512:(sg + 1) * 512])
                mv = work.tile([128, 2], F32, tag="mv", bufs=2,
                               name=f"mv_{mt}")
                nc.vector.bn_aggr(out=mv[:], in_=stats[:])
                rstd = work.tile([128, 1], F32, tag="rstd", bufs=2,
                                 name=f"rstd_{mt}")
                nc.scalar.activation(out=rstd[:], in_=mv[:, 1:2], func=AF.Sqrt,
                                     bias=eps_sb[:], scale=1.0)
                nc.vector.reciprocal(out=rstd[:], in_=rstd[:])
                nc.vector.tensor_scalar(
                    out=osb[:], in0=osb[:], scalar1=mv[:, 0:1],
                    scalar2=rstd[:], op0=ALU.subtract, op1=ALU.mult)
                nc.vector.tensor_tensor(osb[:], osb[:], gamma_bc[:], ALU.mult)
                nc.vector.tensor_tensor(osb[:], osb[:], beta_bc[:], ALU.add)
                nc.sync.dma_start(out_d[mt * 128:(mt + 1) * 128, :], osb[:])

    nc.compile()
    return nc


class _Runner:
    """Compile once; run the SPMD kernel on 8 cores via PJRT repeatedly."""

    def __init__(self):
        self.nc = _build()
        import jax
        from jax.sharding import Mesh, PartitionSpec
        from jax.experimental.shard_map import shard_map
        from concourse import bass2jax
        bass2jax.install_neuronx_cc_hook()

        nc = self.nc
        part_name = (nc.partition_id_tensor.name
                     if nc.partition_id_tensor else None)
        in_names, out_names, out_avals = [], [], []
        for alloc in nc.m.functions[0].allocations:
            if not isinstance(alloc, mybir.MemoryLocationSet):
                continue
            name = alloc.memorylocations[0].name
            if alloc.kind == "ExternalInput":
                if name != part_name:
                    in_names.append(name)
            elif alloc.kind == "ExternalOutput":
                out_names.append(name)
                out_avals.append(jax.core.ShapedArray(
                    tuple(alloc.tensor_shape), mybir.dt.np(alloc.dtype)))
        self.in_names = list(in_names)
        self.out_names = out_names
        self.out_avals = out_avals
        all_in_names = in_names + out_names
        if part_name is not None:
            all_in_names = all_in_names + [part_name]

        def _body(*args):
            operands = list(args)
            if part_name is not None:
                operands.append(bass2jax.partition_id_tensor())
            outs = bass2jax._bass_exec_p.bind(
                *operands, out_avals=tuple(out_avals),
                in_names=tuple(all_in_names), out_names=tuple(out_names),
                lowering_input_output_aliases=(),
                sim_require_finite=True, sim_require_nnan=True, nc=nc)
            return tuple(outs)

        devices = jax.devices()[:N_CORES]
        mesh = Mesh(np.asarray(devices), ("core",))
        nin = len(self.in_names) + len(out_names)
        self.fn = jax.jit(shard_map(
            _body, mesh=mesh, in_specs=(PartitionSpec("core"),) * nin,
            out_specs=(PartitionSpec("core"),) * len(out_names),
            check_rep=False))
        self.jax = jax

    def stage(self, in_maps):
        """Concatenate per-core inputs + zero outputs, device_put once."""
        concat = [np.concatenate([m[name] for m in in_maps], axis=0)
                  for name in self.in_names]
        zeros = [np.zeros((N_CORES * a.shape[0], *a.shape[1:]), a.dtype)
                 for a in self.out_avals]
        return [self.jax.device_put(x) for x in concat + zeros]

    def run_staged(self, staged):
        outs = self.fn(*staged)
        self.jax.block_until_ready(outs)
        return outs

    def run(self, in_maps):
        outs = self.run_staged(self.stage(in_maps))
        return [
            {name: np.asarray(outs[i]).reshape(
                N_CORES, *self.out_avals[i].shape)[c]
             for i, name in enumerate(self.out_names)}
            for c in range(N_CORES)
        ]


_RUNNER = None


def _get_runner():
    global _RUNNER
    if _RUNNER is None:
        _RUNNER = _Runner()
    return _RUNNER


def _make_in_maps(x, w_qkv, w_out, b_out, ln_gamma, ln_beta):
    x = np.asarray(x, dtype=np.float32)
    w_qkv = np.asarray(w_qkv, dtype=np.float32)
    w_out = np.asarray(w_out, dtype=np.float32)
    b_out = np.asarray(b_out, dtype=np.float32)
    ln_gamma = np.asarray(ln_gamma, dtype=np.float32)
    ln_beta = np.asarray(ln_beta, dtype=np.float32)

    xT = np.ascontiguousarray(x.reshape(ROWS, DIM).T)
    in_maps = []
    for c in range(N_CORES):
        h0 = HPC * c * DH
        cols = np.concatenate([
            w_qkv[:, h0:h0 + HPC * DH],
            w_qkv[:, DIM + h0:DIM + h0 + HPC * DH],
            w_qkv[:, 2 * DIM + h0:2 * DIM + h0 + HPC * DH],
        ], axis=1)
        in_maps.append({
            "xT": xT,
            "wqkv": np.ascontiguousarray(cols),
            "wout": w_out,
            "bout": b_out,
            "gamma": ln_gamma,
            "beta": ln_beta,
        })
    return in_maps


def _unshard_out(stacked):
    """[N_CORES*RPC, DIM] row-stacked per-core outputs -> [B, N, DIM]."""
    return stacked.reshape(B, N, DIM).astype(np.float32)


def kernel(x, w_qkv, w_out, b_out, ln_gamma, ln_beta):
    runner = _get_runner()
    in_maps = _make_in_maps(x, w_qkv, w_out, b_out, ln_gamma, ln_beta)
    results = runner.run(in_maps)
    out = np.concatenate([results[c]["out"] for c in range(N_CORES)], axis=0)
    return _unshard_out(out)

